# revision 52
# baseline (speedup 1.0000x reference)
"""Trainium2 Bass kernel for the gnn_message_passing LoopModel.

Reference computation (per edge e, corners l/r from edge_corner):
    CF[n]    = mean over pairs (n, e') of x[e']          (segment mean)
    out[e]   = relu(W1 @ x[e] + W2 @ CF[l_e] + W3 @ CF[r_e] + W4 @ max_e x)

Distribution over 8 NeuronCores (436us baseline -> ~270us):
  - corner table sharded 32 corners/core (host balances incident-pair load);
    scatter stage = fp8 dma_gather of incident x rows (big 8/32-channel
    elements to cut SWDGE descriptor count) + matmuls against a host-built
    scatter matrix (1/count folded in), 4 corner-subtiles packed per PSUM
    bank via tile_position -> fp8 table slice [2048 rows, 1024 cols
    (784 hw + pad; gather elem/stride must be a multiple of 256B)]
  - the per-core bf16 local max rides the SAME fp8 AllGather as 128 extra
    byte-packed rows (no separate AllReduce); consumers reduce the 8 core
    maxes with a parallel-load tree of vector max ops
  - ONE AllGather replicates the table; its gpsimd doorbell blocks the
    queue until completion, so phase-4 corner-gather descriptor generation
    is split around it: NPRE preps before the bell (generated during the
    phase-1 tail), trigger #1 fires them the instant the AG lands, the
    rest generate right after.  prepare_only preps defer their table-read
    dependency to the trigger; consumers take explicit semaphore waits
    pinned after the trigger (tile does not auto-wire RAW on prepped DMAs).
  - conv stage edge-sharded 64 edges/core, 2-edge batched: per PSUM tile
    one fp8 DoubleRow matmul folds W2 @ CF[l] + W3 @ CF[r] (stacked
    k-tiles), then bf16 W1 @ x (x stays SBUF-resident from one flat
    host-shuffled load) and W4 @ gmax accumulate; scalar relu streams to
    the output.
"""

import os
import sys
import numpy as np

for _p in ("/opt/trn_rl_repo", "/root/.axon_site/_ro/trn_rl_repo"):
    if os.path.isdir(_p) and _p not in sys.path:
        sys.path.insert(0, _p)

import ml_dtypes  # noqa: E402
from concourse import bacc, bass, mybir, tile  # noqa: E402
from concourse.bass_utils import run_bass_kernel_spmd  # noqa: E402

N_CORES = 8
E, C, H, W = 512, 64, 28, 28
HW = H * W                      # 784
CH0, CH1 = 512, 272             # hw split: chunk0 cols, chunk1 cols
TCOLS = 1024                    # table row length (784 real + 240 pad)
NC_TOT = 256                    # corner table rows (padded if num_corners < 256)
N_LOC = NC_TOT // N_CORES       # 32 corners per core
E_LOC = E // N_CORES            # 64 edges per core
S0R = N_LOC * C                 # 2048 table rows per core slice
GMR = 128                       # gmax rows appended to the slice (bf16 bytes)
SROWS = S0R + GMR               # 2176 rows per core slice

BF16 = ml_dtypes.bfloat16
FP8 = ml_dtypes.float8_e4m3

_PROGRAM_CACHE = {}


# --------------------------------------------------------------------------
# host-side helpers
# --------------------------------------------------------------------------

def _round_bf16(a):
    """fp32 -> bf16 (round to nearest even), returned as uint16."""
    v = np.ascontiguousarray(a, dtype=np.float32).view(np.uint32)
    return ((v + 0x7FFF + ((v >> 16) & 1)) >> 16).astype(np.uint16)


def _balance_corners(counts):
    """Assign NC_TOT corners to N_CORES bins, N_LOC corners per bin,
    minimizing the max total incident-pair count per bin."""
    order = np.argsort(-counts, kind="stable")
    loads = np.zeros(N_CORES, dtype=np.int64)
    slots = np.zeros(N_CORES, dtype=np.int64)
    assign = np.full(NC_TOT, -1, dtype=np.int64)
    for c in order:
        cand = [b for b in range(N_CORES) if slots[b] < N_LOC]
        b = min(cand, key=lambda i: (loads[i], slots[i]))
        assign[c] = b
        loads[b] += counts[c]
        slots[b] += 1
    target = counts.sum() // N_CORES
    for _ in range(4096):
        hi = int(np.argmax(loads))
        lo = int(np.argmin(loads))
        if loads[hi] <= max(target, 128):
            break
        best = None
        ch = np.where(assign == hi)[0]
        cl = np.where(assign == lo)[0]
        for a in ch:
            for b2 in cl:
                d = counts[a] - counts[b2]
                if 0 < d <= loads[hi] - loads[lo]:
                    if best is None or abs(d - (loads[hi] - target)) < abs(
                        best[2] - (loads[hi] - target)
                    ):
                        best = (a, b2, d)
        if best is None:
            break
        a, b2, d = best
        assign[a], assign[b2] = lo, hi
        loads[hi] -= d
        loads[lo] += d
    return assign, loads


def _wrap_idxs(idx_flat, n_pad):
    """Pack flat gather indices into the dma_gather wrapped layout:
    [128, n_pad//16] int16 with logical index i at [i%16, i//16],
    replicated across the 8 groups of 16 partitions."""
    assert n_pad % 16 == 0
    w = np.zeros((16, n_pad // 16), dtype=np.int16)
    for i, v in enumerate(idx_flat):
        w[i % 16, i // 16] = v
    return np.tile(w, (8, 1))


def _prepare(x, W_agg, corner_edge_pairs, edge_corner, num_corners):
    x = np.asarray(x, dtype=np.float32)
    W_agg = np.asarray(W_agg, dtype=np.float32)
    cep = np.asarray(corner_edge_pairs).astype(np.int64)
    ec = np.asarray(edge_corner).astype(np.int64)
    ncorn = int(num_corners)
    assert x.shape == (E, C, H, W), x.shape
    assert ncorn <= NC_TOT

    # reference semantics: scatter drops out-of-range segments, gathers clamp
    seg = cep[:, 0]
    eid = np.clip(cep[:, 1], 0, E - 1)
    valid = (seg >= 0) & (seg < ncorn)
    seg_v, eid_v = seg[valid], eid[valid]
    ec_cl = np.clip(ec, 0, max(ncorn - 1, 0))

    counts = np.bincount(seg_v, minlength=NC_TOT).astype(np.int64)
    inv_count = 1.0 / np.maximum(counts, 1).astype(np.float64)

    assign, loads = _balance_corners(counts)
    k_chunks = max(1, int(-(-int(loads.max()) // 128)))  # ceil(maxload/128)
    k_pad = 128 * k_chunks

    # permuted corner position in the all-gathered table
    pos = np.zeros(NC_TOT, dtype=np.int64)
    slot_ctr = np.zeros(N_CORES, dtype=np.int64)
    for c in range(NC_TOT):
        b = assign[c]
        pos[c] = b * N_LOC + slot_ctr[b]
        slot_ctr[b] += 1

    # per-core incident pair lists
    pair_eids = [[] for _ in range(N_CORES)]
    pair_local = [[] for _ in range(N_CORES)]
    pair_inv = [[] for _ in range(N_CORES)]
    for p in range(len(seg_v)):
        c = int(seg_v[p])
        b = int(assign[c])
        pair_eids[b].append(int(eid_v[p]))
        pair_local[b].append(int(pos[c] - b * N_LOC))
        pair_inv[b].append(inv_count[c])

    # x in bf16 (local conv/max input) and fp8 (scatter gather source)
    xr = x.reshape(E, C, HW)
    xb = _round_bf16(xr)                                  # [E, 64, 784] u16
    x8 = np.clip(xr, -240.0, 240.0).astype(FP8)           # [E, 64, 784] fp8
    # bf16 local layouts, pre-shuffled so each core's chunk is one flat
    # [128, 32*CH] copy: partition p = (edge%2)*64 + ch, col = je*CH + d
    def _shuffle_local(xc):
        CH = xc.shape[2]
        # [E, 64, CH] -> per core [32 je, 2 m, 64 ch, CH] -> [(m ch), (je d)]
        a = xc.reshape(N_CORES, E_LOC // 2, 2, C, CH)
        a = a.transpose(0, 2, 3, 1, 4)            # [b, m, ch, je, d]
        return np.ascontiguousarray(a).reshape(N_CORES * 128, (E_LOC // 2) * CH)

    xl0 = _shuffle_local(xb[:, :, :CH0])
    xl1 = _shuffle_local(xb[:, :, CH0:])
    # fp8 gather layouts (big rows: fewer SWDGE descriptors)
    xf0 = np.ascontiguousarray(x8[:, :, :CH0]).reshape(E * 4, 16 * CH0)
    xf1 = np.ascontiguousarray(x8[:, :, CH0:]).reshape(E, 64 * CH1)

    # block-diagonal weights for 2-edge batched conv matmuls
    wblk = np.zeros((4, 128, 128), dtype=np.float32)
    for t in range(4):
        wt = W_agg[:, t * 64:(t + 1) * 64].T          # [c, o]
        wblk[t, :64, :64] = wt
        wblk[t, 64:, 64:] = wt
    wblk_in = _round_bf16(wblk.reshape(4 * 128, 128))
    # W2/W3 block-diag stacked on the k-tile dim, fp8 for DoubleRow matmul
    w23 = np.stack([wblk[1], wblk[2]], axis=1)            # [128, 2, 128]
    w23_8 = np.clip(w23, -240.0, 240.0).astype(FP8).reshape(256, 128)
    ident = _round_bf16(np.eye(128, dtype=np.float32))

    per_core = []
    for b in range(N_CORES):
        k_real = len(pair_eids[b])
        assert k_real <= k_pad
        eids_b = np.zeros(k_pad, dtype=np.int64)
        eids_b[:k_real] = pair_eids[b]
        mc = np.zeros((k_pad, N_LOC), dtype=np.float32)
        for p in range(k_real):
            mc[p, pair_local[b][p]] += pair_inv[b][p]

        # stage-1 gather indices
        # chunk0: per (kc, j2 in 4): 128 idxs, idx = eid*4 + j2 (16 chs/row)
        # chunk1: per kc: 128 idxs, idx = eid (64 chs/row)
        s10_cols = []
        s11_cols = []
        for kc in range(k_chunks):
            epk = eids_b[kc * 128:(kc + 1) * 128]
            for j2 in range(4):
                s10_cols.append(_wrap_idxs((epk * 4 + j2).astype(np.int16), 128))
            s11_cols.append(_wrap_idxs(epk.astype(np.int16), 128))
        s10 = np.concatenate(s10_cols, axis=1)   # [128, 8*4*k_chunks]
        s11 = np.concatenate(s11_cols, axis=1)   # [128, 8*k_chunks]

        # stage-4 gather indices: per binstr, 1024 idxs into the ag table
        # i = s*128 + m*64 + ch -> table row of corner(edge, t), channel ch
        e0 = b * E_LOC
        s4_cols = []
        ch64 = np.arange(64, dtype=np.int64)
        for binstr in range(8):
            f = np.zeros(1024, dtype=np.int64)
            for ep in range(4):
                for t in range(2):
                    s = ep * 2 + t
                    for m in range(2):
                        le = binstr * 8 + ep * 2 + m
                        p_c = pos[int(ec_cl[e0 + le, t])]
                        ob, sl = p_c // N_LOC, p_c % N_LOC
                        i0 = s * 128 + m * 64
                        f[i0:i0 + 64] = ob * SROWS + sl * 64 + ch64
            s4_cols.append(_wrap_idxs(f.astype(np.int16), 1024))
        s4 = np.concatenate(s4_cols, axis=1)     # [128, 512]

        per_core.append(dict(mc=_round_bf16(mc), s10=s10, s11=s11, s4=s4))

    return xl0, xl1, xf0, xf1, wblk_in, w23_8, ident, per_core, k_chunks


# --------------------------------------------------------------------------
# device program
# --------------------------------------------------------------------------

def _build_program(k_chunks):
    dbg_no_prep = os.environ.get("DBG_NO_PREP", "0") == "1"
    dbg_zero_table = os.environ.get("DBG_ZERO_TABLE", "0") == "1"
    dbg_zero_xw = os.environ.get("DBG_ZERO_XW", "0") == "1"
    bf = mybir.dt.bfloat16
    f8 = mybir.dt.float8e4
    f32 = mybir.dt.float32
    i16 = mybir.dt.int16

    nc = bacc.Bacc("TRN2", target_bir_lowering=False, debug=False,
                   num_devices=N_CORES, num_swdge_queues=2)

    xf0_t = nc.dram_tensor("xf0", [E * 4, 16 * CH0], f8, kind="ExternalInput").ap()
    xf1_t = nc.dram_tensor("xf1", [E, 64 * CH1], f8, kind="ExternalInput").ap()
    xl0_t = nc.dram_tensor("xl0", [128, 32 * CH0], bf, kind="ExternalInput").ap()
    xl1_t = nc.dram_tensor("xl1", [128, 32 * CH1], bf, kind="ExternalInput").ap()
    wb_t = nc.dram_tensor("wb", [4 * 128, 128], bf, kind="ExternalInput").ap()
    w23_t = nc.dram_tensor("w23", [256, 128], f8, kind="ExternalInput").ap()
    id_t = nc.dram_tensor("ident", [128, 128], bf, kind="ExternalInput").ap()
    mc_t = nc.dram_tensor("mc", [128 * k_chunks, N_LOC], bf, kind="ExternalInput").ap()
    s10_t = nc.dram_tensor("s10", [128, 8 * 4 * k_chunks], i16, kind="ExternalInput").ap()
    s11_t = nc.dram_tensor("s11", [128, 8 * k_chunks], i16, kind="ExternalInput").ap()
    s4_t = nc.dram_tensor("s4", [128, 512], i16, kind="ExternalInput").ap()
    out_t = nc.dram_tensor("outp", [E_LOC * C, HW], bf, kind="ExternalOutput").ap()
    dbg_dump = os.environ.get("DBG_DUMP_TABLE", "0") != "0"
    if dbg_dump:
        dump_t = nc.dram_tensor("dump", [N_CORES * SROWS, TCOLS], mybir.dt.uint8,
                                kind="ExternalOutput").ap()

    with tile.TileContext(nc) as tc:
        with tc.tile_pool(name="dram", bufs=1, space="DRAM") as dram, \
             tc.tile_pool(name="consts", bufs=1) as consts, \
             tc.tile_pool(name="xw", bufs=1) as xwp:
            slice_t = dram.tile([SROWS, TCOLS], f8)
            ag_t = dram.tile([N_CORES * SROWS, TCOLS], f8, addr_space="Shared")

            # constants (one DMA each)
            wtile = consts.tile([128, 4, 128], bf, tag="wt")
            nc.sync.dma_start(out=wtile[:],
                              in_=wb_t[:].rearrange("(t p) o -> p t o", t=4))
            w23tile = consts.tile([128, 2, 128], f8, tag="w23")
            nc.sync.dma_start(out=w23tile[:],
                              in_=w23_t[:].rearrange("(p t) o -> p t o", t=2))
            mctiles = []
            for kc in range(k_chunks):
                mt = consts.tile([128, N_LOC], bf, tag=f"mc{kc}")
                nc.sync.dma_start(out=mt[:], in_=mc_t[kc * 128:(kc + 1) * 128, :])
                mctiles.append(mt)
            s10tile = consts.tile([128, 8 * 4 * k_chunks], i16, tag="s10")
            nc.sync.dma_start(out=s10tile[:], in_=s10_t[:])
            s11tile = consts.tile([128, 8 * k_chunks], i16, tag="s11")
            nc.sync.dma_start(out=s11tile[:], in_=s11_t[:])
            s4tile = consts.tile([128, 512], i16, tag="s4")
            nc.sync.dma_start(out=s4tile[:], in_=s4_t[:])
            gm0 = consts.tile([128, CH0], bf, tag="gm0")
            gm1 = consts.tile([128, CH1], bf, tag="gm1")

            # zero the table pad columns (cols HW..TCOLS) and gmax rows once
            PAD = TCOLS - HW
            zt = consts.tile([128, (S0R // 128) * PAD], mybir.dt.float8e4,
                             tag="zt")
            nc.vector.memset(zt[:], 0.0)
            nc.sync.dma_start(
                out=slice_t[0:S0R, :].rearrange("(g r) d -> r g d", r=128)
                [:, :, HW:TCOLS],
                in_=zt[:],
            )
            nc.sync.dma_start(out=slice_t[S0R:SROWS, :],
                              in_=zt[:, 0:TCOLS])

            # x stays resident in SBUF; phase 4 reads it as matmul rhs
            xb0 = xwp.tile([128, 32 * CH0], bf, tag="xb0")
            xb1 = xwp.tile([128, 32 * CH1], bf, tag="xb1")

            def w(t):
                return wtile[:, t, :]

            # phase-4 gather tiles: own up-front pool so the prepped DMA
            # writes never land in SBUF space being reused by other pools
            lrts = []
            for binstr in range(8):
                lrt = xwp.tile([128, 8, TCOLS], f8, tag=f"lrt{binstr}")
                lrts.append(lrt)

            with tc.tile_pool(name="p2", bufs=1) as p2, \
                 tc.tile_pool(name="p2r", bufs=8) as p2r, \
                 tc.tile_pool(name="p1", bufs=4) as p1, \
                 tc.tile_pool(name="p1b", bufs=1) as p1b, \
                 tc.tile_pool(name="p1s", bufs=6) as p1s, \
                 tc.tile_pool(name="psum1", bufs=3, space="PSUM") as psum1, \
                 tc.tile_pool(name="psum1b", bufs=5, space="PSUM") as psum1b:
                mx0 = p2.tile([128, CH0], bf, tag="mx0")
                mx1 = p2.tile([128, CH1], bf, tag="mx1")

                def scatter0(j2):
                    gts = []
                    for kc in range(k_chunks):
                        gt = p1.tile([128, 1, 16 * CH0], f8, tag="gt0")
                        nc.gpsimd.dma_gather(
                            gt[:], xf0_t[:],
                            s10tile[:, (kc * 4 + j2) * 8:(kc * 4 + j2) * 8 + 8],
                            num_idxs=128, num_idxs_reg=128, elem_size=16 * CH0,
                        )
                        gts.append(gt)
                    for t in range(4):
                        ps = psum1.tile([128, CH0], f32, space="PSUM", tag="ps1a")
                        for i in range(4):
                            q = t * 4 + i
                            for kc in range(k_chunks):
                                nc.tensor.matmul(
                                    out=ps[32 * i:32 * (i + 1), :],
                                    lhsT=mctiles[kc][:],
                                    rhs=gts[kc][:, 0, q * CH0:(q + 1) * CH0],
                                    start=(kc == 0), stop=(kc == k_chunks - 1),
                                    tile_position=(0, 32 * i),
                                )
                        stg = p1s.tile([128, CH0], f8, tag="stg0")
                        nc.scalar.activation(stg[:], ps[:],
                                             mybir.ActivationFunctionType.Copy)
                        nc.sync.dma_start(
                            out=slice_t[0:S0R, :]
                            .rearrange("(c h) d -> h c d", c=N_LOC)
                            [16 * j2 + 4 * t:16 * j2 + 4 * t + 4, :, 0:CH0],
                            in_=stg[:],
                        )

                def scatter1():
                    gts = []
                    for kc in range(k_chunks):
                        gt = p1b.tile([128, 1, 64 * CH1], f8, tag="gt1")
                        nc.gpsimd.dma_gather(
                            gt[:], xf1_t[:],
                            s11tile[:, kc * 8:kc * 8 + 8],
                            num_idxs=128, num_idxs_reg=128, elem_size=64 * CH1,
                        )
                        gts.append(gt)
                    for t in range(16):
                        ps = psum1b.tile([128, CH1], f32, space="PSUM", tag="ps1b")
                        for i in range(4):
                            q = t * 4 + i
                            for kc in range(k_chunks):
                                nc.tensor.matmul(
                                    out=ps[32 * i:32 * (i + 1), :],
                                    lhsT=mctiles[kc][:],
                                    rhs=gts[kc][:, 0, q * CH1:(q + 1) * CH1],
                                    start=(kc == 0), stop=(kc == k_chunks - 1),
                                    tile_position=(0, 32 * i),
                                )
                        stg = p1s.tile([128, CH1], f8, tag="stg1")
                        nc.scalar.activation(stg[:], ps[:],
                                             mybir.ActivationFunctionType.Copy)
                        nc.sync.dma_start(
                            out=slice_t[0:S0R, :]
                            .rearrange("(c h) d -> h c d", c=N_LOC)
                            [4 * t:4 * t + 4, :, CH0:CH0 + CH1],
                            in_=stg[:],
                        )

                # x loads up front; chunk1 scatter (independent of x)
                # interleaved mid-stream; max chains ride the vector queue
                nc.scalar.dma_start(out=xb0[:], in_=xl0_t[:])
                nc.scalar.dma_start(out=xb1[:], in_=xl1_t[:])
                scatter1()
                scatter0(0)
                scatter0(1)
                scatter0(2)
                scatter0(3)
                for chunk, mx, xc, CH in ((0, mx0, xb0, CH0), (1, mx1, xb1, CH1)):
                    for j in range(32):
                        sl = xc[:, j * CH:(j + 1) * CH]
                        if j == 0:
                            nc.vector.tensor_copy(out=mx[:], in_=sl)
                        else:
                            nc.vector.tensor_tensor(out=mx[:], in0=mx[:],
                                                    in1=sl,
                                                    op=mybir.AluOpType.max)

                # ---- local max fold -> gmax rows of the table slice -------
                half0 = p2.tile([64, CH0], bf, tag="h0")
                nc.scalar.dma_start(out=half0[:], in_=mx0[64:128, :])
                nc.vector.tensor_tensor(out=mx0[0:64, :], in0=mx0[0:64, :],
                                        in1=half0[:], op=mybir.AluOpType.max)
                half1 = p2.tile([64, CH1], bf, tag="h1")
                nc.scalar.dma_start(out=half1[:], in_=mx1[64:128, :])
                nc.vector.tensor_tensor(out=mx1[0:64, :], in0=mx1[0:64, :],
                                        in1=half1[:], op=mybir.AluOpType.max)
                # gmax bf16 bytes ride the fp8 table AllGather: per channel c,
                # row S0R+2c = cols 0:511 (1024B), row S0R+2c+1 = cols 512:783
                nc.sync.dma_start(
                    out=slice_t[S0R:SROWS, :]
                    .rearrange("(c two) d -> c (two d)", two=2)[:, 0:2 * CH0],
                    in_=mx0[0:64, :].bitcast(f8),
                )
                nc.sync.dma_start(
                    out=slice_t[S0R:SROWS, :]
                    .rearrange("(c two) d -> c (two d)", two=2)
                    [:, TCOLS:TCOLS + 2 * CH1],
                    in_=mx1[0:64, :].bitcast(f8),
                )

                # ---- phase-4 gather preps + AllGather, all on gpsimd:
                # NPRE preps generate during the phase-1 tail, then the AG
                # doorbell rings (it blocks gpsimd until the AG completes),
                # trigger #1 fires the first wave the moment the AG lands,
                # and the remaining preps + trigger #2 follow ---------------
                NPRE = 8
                p4sems = []
                preps = []

                def emit_prep(binstr):
                    dma_sem = nc.alloc_semaphore(f"p4g{binstr}")
                    pr = nc.gpsimd.dma_gather(
                        lrts[binstr][:], ag_t[:],
                        s4tile[:, binstr * 64:(binstr + 1) * 64],
                        num_idxs=1024, num_idxs_reg=1024, elem_size=TCOLS,
                        prepare_only=True, sem=dma_sem, queue_num=1,
                    )
                    preps.append(pr)
                    p4sems.append(dma_sem)

                if not dbg_no_prep:
                    for binstr in range(NPRE):
                        emit_prep(binstr)
                agi = nc.gpsimd.collective_compute(
                    "AllGather", mybir.AluOpType.bypass,
                    replica_groups=[list(range(N_CORES))],
                    ins=[slice_t.opt()], outs=[ag_t.opt()],
                )
                trig = None
                if not dbg_no_prep:
                    for pr in preps:
                        bass._add_dep_helper(agi.ins, pr.ins, False,
                                             "pin AG bell after prep wave 1")
                    trig = nc.gpsimd.trigger_dma(count=None, queue_num=1)
                    bass._add_dep_helper(trig.ins, agi.ins, True,
                                         "gather DMA fires after AllGather")
                    if NPRE < 8:
                        for binstr in range(NPRE, 8):
                            emit_prep(binstr)
                            bass._add_dep_helper(preps[-1].ins, agi.ins, False,
                                                 "pin prep wave 2 after AG bell")
                        trig2 = nc.gpsimd.trigger_dma(count=None, queue_num=1)
                        bass._add_dep_helper(trig2.ins, agi.ins, True,
                                             "gather DMA fires after AllGather")

                # ---- global max: parallel block loads + tree reduce -------
                t0s, t1s = [], []
                for b in range(N_CORES):
                    gv = ag_t[b * SROWS + S0R:(b * SROWS) + SROWS, :] \
                        .rearrange("(c two) d -> c (two d)", two=2)
                    t0 = p2r.tile([64, CH0], bf, tag="t0")
                    ld0 = nc.sync.dma_start(out=t0[:],
                                            in_=gv[:, 0:2 * CH0].bitcast(bf))
                    bass._add_dep_helper(ld0.ins, agi.ins, True,
                                         "gmax block read after AllGather")
                    t1 = p2r.tile([64, CH1], bf, tag="t1")
                    ld1 = nc.sync.dma_start(
                        out=t1[:],
                        in_=gv[:, TCOLS:TCOLS + 2 * CH1].bitcast(bf))
                    bass._add_dep_helper(ld1.ins, agi.ins, True,
                                         "gmax block read after AllGather")
                    t0s.append(t0)
                    t1s.append(t1)
                for ts in (t0s, t1s):
                    for stride in (4, 2, 1):
                        for i in range(stride):
                            nc.vector.tensor_tensor(
                                out=ts[i][:], in0=ts[i][:],
                                in1=ts[i + stride][:],
                                op=mybir.AluOpType.max)
                nc.sync.dma_start(out=gm0[0:64, :], in_=t0s[0][:])
                nc.sync.dma_start(out=gm0[64:128, :], in_=t0s[0][:])
                nc.sync.dma_start(out=gm1[0:64, :], in_=t1s[0][:])
                nc.sync.dma_start(out=gm1[64:128, :], in_=t1s[0][:])

            if dbg_dump:
                if os.environ.get("DBG_DUMP_TABLE") == "2":
                    nc.sync.dma_start(out=dump_t[0:S0R, :],
                                      in_=slice_t[:].bitcast(mybir.dt.uint8))
                else:
                    nc.sync.dma_start(out=dump_t[:],
                                      in_=ag_t[:].bitcast(mybir.dt.uint8))

            # ---- phase 4: prep gathers during AG, trigger after -----------
            with tc.tile_pool(name="p4o", bufs=4) as p4o, \
                 tc.tile_pool(name="psum4", bufs=4, space="PSUM") as psum4:
                if dbg_no_prep:
                    for binstr in range(8):
                        gi = nc.gpsimd.dma_gather(
                            lrts[binstr][:], ag_t[:],
                            s4tile[:, binstr * 64:(binstr + 1) * 64],
                            num_idxs=1024, num_idxs_reg=1024, elem_size=TCOLS,
                        )
                        bass._add_dep_helper(gi.ins, agi.ins, True,
                                             "gather reads table after AG")

                for binstr in range(8):
                    lrt = lrts[binstr]
                    wt = None
                    if trig is not None:
                        wt = nc.tensor.wait_ge(p4sems[binstr], 16)
                        bass._add_dep_helper(
                            wt.ins, trig.ins, True,
                            "phase4 lrt consume waits gather DMA (post-trigger)")
                    ot0 = p4o.tile([128, 4, CH0], bf, tag="ot0")
                    ot1 = p4o.tile([128, 4, CH1], bf, tag="ot1")
                    for ep in range(4):
                        je = binstr * 4 + ep
                        for half in range(2):
                            CH = CH0 if half == 0 else CH1
                            c0 = 0 if half == 0 else CH0
                            xk = xb0 if half == 0 else xb1
                            ot = ot0 if half == 0 else ot1
                            ps = psum4.tile([128, CH], f32, space="PSUM",
                                            tag=f"ps4{half}")
                            gm = gm0 if half == 0 else gm1
                            mms = []
                            if not dbg_zero_table:
                                # W2 @ CF[l] + W3 @ CF[r]: one fp8 DoubleRow
                                # matmul over the 2 stacked k-tiles
                                mms.append((dict(
                                    lhsT=w23tile[:],
                                    rhs=lrt[:, ep * 2:ep * 2 + 2, c0:c0 + CH],
                                    perf_mode=mybir.MatmulPerfMode.DoubleRow),
                                    True))
                            if not dbg_zero_xw:
                                mms.append((dict(
                                    lhsT=w(0),
                                    rhs=xk[:, je * CH:(je + 1) * CH]),
                                    False))
                                mms.append((dict(lhsT=w(3), rhs=gm[:]), False))
                            for k, (kw, is_lrt) in enumerate(mms):
                                mm = nc.tensor.matmul(
                                    out=ps[:], start=(k == 0),
                                    stop=(k == len(mms) - 1), **kw)
                                if wt is not None and is_lrt:
                                    bass._add_dep_helper(
                                        mm.ins, wt.ins, True,
                                        "mm reads lrt after DMA wait")
                            nc.scalar.activation(
                                ot[:, ep, :], ps[:],
                                mybir.ActivationFunctionType.Relu)

                    # out row (binstr*8 + ep*2)*64 + p = bi*512 + ep*128 + p
                    nc.sync.dma_start(
                        out=out_t[:]
                        .rearrange("(bi ep p) d -> bi p ep d", bi=8, ep=4)
                        [binstr, :, :, 0:CH0],
                        in_=ot0[:],
                    )
                    nc.sync.dma_start(
                        out=out_t[:]
                        .rearrange("(bi ep p) d -> bi p ep d", bi=8, ep=4)
                        [binstr, :, :, CH0:HW],
                        in_=ot1[:],
                    )


    nc.compile()
    return nc


# --------------------------------------------------------------------------
# entry point
# --------------------------------------------------------------------------

def _run(x, W_agg, corner_edge_pairs, edge_corner, num_corners,
         trace=False):
    xl0, xl1, xf0, xf1, wblk_in, w23_8, ident, per_core, k_chunks = _prepare(
        x, W_agg, corner_edge_pairs, edge_corner, num_corners)

    key = (k_chunks, os.environ.get("DBG_NO_PREP"),
           os.environ.get("DBG_ZERO_TABLE"), os.environ.get("DBG_ZERO_XW"))
    if key not in _PROGRAM_CACHE:
        _PROGRAM_CACHE[key] = _build_program(k_chunks)
    nc = _PROGRAM_CACHE[key]

    xl0_b = xl0.view(BF16)
    xl1_b = xl1.view(BF16)
    in_maps = []
    for b in range(N_CORES):
        pc = per_core[b]
        in_maps.append({
            "xf0": xf0, "xf1": xf1,
            "xl0": xl0_b[b * 128:(b + 1) * 128],
            "xl1": xl1_b[b * 128:(b + 1) * 128],
            "wb": wblk_in.view(BF16), "w23": w23_8, "ident": ident.view(BF16),
            "mc": pc["mc"].view(BF16),
            "s10": pc["s10"], "s11": pc["s11"], "s4": pc["s4"],
        })

    kwargs = {}
    if trace:
        kwargs = dict(trace=True, trace_cores=list(range(N_CORES)))
    res = run_bass_kernel_spmd(nc, in_maps, list(range(N_CORES)), **kwargs)

    out = np.empty((E, C, HW), dtype=np.float32)
    for b in range(N_CORES):
        o = np.asarray(res.results[b]["outp"]).view(np.uint16)
        f = (o.astype(np.uint32) << 16).view(np.float32).reshape(E_LOC, C, HW)
        out[b * E_LOC:(b + 1) * E_LOC] = f
    return out.reshape(E, C, H, W), res


def kernel(x, W_agg, corner_edge_pairs, edge_corner, num_corners):
    out, _ = _run(x, W_agg, corner_edge_pairs, edge_corner, num_corners,
                  trace=False)
    return out


# expose for test harness profiling
def _run_profiled(x, W_agg, corner_edge_pairs, edge_corner, num_corners,
                  trace=True):
    return _run(x, W_agg, corner_edge_pairs, edge_corner, num_corners,
                trace=trace)


# revision 53
# speedup vs baseline: 1.0877x; 1.0877x over previous
"""Trainium2 Bass kernel for the gnn_message_passing LoopModel.

Reference computation (per edge e, corners l/r from edge_corner):
    CF[n]    = mean over pairs (n, e') of x[e']          (segment mean)
    out[e]   = relu(W1 @ x[e] + W2 @ CF[l_e] + W3 @ CF[r_e] + W4 @ max_e x)

Distribution over 8 NeuronCores (436us baseline -> ~270us):
  - corner table sharded 32 corners/core (host balances incident-pair load);
    scatter stage = fp8 dma_gather of incident x rows (big 8/32-channel
    elements to cut SWDGE descriptor count) + matmuls against a host-built
    scatter matrix (1/count folded in), 4 corner-subtiles packed per PSUM
    bank via tile_position -> fp8 table slice [2048 rows, 1024 cols
    (784 hw + pad; gather elem/stride must be a multiple of 256B)]
  - the per-core bf16 local max rides the SAME fp8 AllGather as 128 extra
    byte-packed rows (no separate AllReduce); consumers reduce the 8 core
    maxes with a parallel-load tree of vector max ops
  - ONE AllGather replicates the table; its gpsimd doorbell blocks the
    queue until completion, so phase-4 corner-gather descriptor generation
    is split around it: NPRE preps before the bell (generated during the
    phase-1 tail), trigger #1 fires them the instant the AG lands, the
    rest generate right after.  prepare_only preps defer their table-read
    dependency to the trigger; consumers take explicit semaphore waits
    pinned after the trigger (tile does not auto-wire RAW on prepped DMAs).
  - conv stage edge-sharded 64 edges/core, 2-edge batched: per PSUM tile
    one fp8 DoubleRow matmul folds W2 @ CF[l] + W3 @ CF[r] (stacked
    k-tiles), then bf16 W1 @ x (x stays SBUF-resident from one flat
    host-shuffled load) and W4 @ gmax accumulate; scalar relu streams to
    the output.
"""

import os
import sys
import numpy as np

for _p in ("/opt/trn_rl_repo", "/root/.axon_site/_ro/trn_rl_repo"):
    if os.path.isdir(_p) and _p not in sys.path:
        sys.path.insert(0, _p)

import ml_dtypes  # noqa: E402
from concourse import bacc, bass, mybir, tile  # noqa: E402
from concourse.bass_utils import run_bass_kernel_spmd  # noqa: E402

N_CORES = 8
E, C, H, W = 512, 64, 28, 28
HW = H * W                      # 784
CH0, CH1 = 512, 272             # hw split: chunk0 cols, chunk1 cols
TCOLS = 1024                    # table row length (784 real + 240 pad)
NC_TOT = 256                    # corner table rows (padded if num_corners < 256)
N_LOC = NC_TOT // N_CORES       # 32 corners per core
E_LOC = E // N_CORES            # 64 edges per core
S0R = N_LOC * C                 # 2048 table rows per core slice
GMR = 128                       # gmax rows appended to the slice (bf16 bytes)
SROWS = S0R + GMR               # 2176 rows per core slice

BF16 = ml_dtypes.bfloat16
FP8 = ml_dtypes.float8_e4m3

_PROGRAM_CACHE = {}


# --------------------------------------------------------------------------
# host-side helpers
# --------------------------------------------------------------------------

def _round_bf16(a):
    """fp32 -> bf16 (round to nearest even), returned as uint16."""
    v = np.ascontiguousarray(a, dtype=np.float32).view(np.uint32)
    return ((v + 0x7FFF + ((v >> 16) & 1)) >> 16).astype(np.uint16)


def _balance_corners(counts):
    """Assign NC_TOT corners to N_CORES bins, N_LOC corners per bin,
    minimizing the max total incident-pair count per bin."""
    order = np.argsort(-counts, kind="stable")
    loads = np.zeros(N_CORES, dtype=np.int64)
    slots = np.zeros(N_CORES, dtype=np.int64)
    assign = np.full(NC_TOT, -1, dtype=np.int64)
    for c in order:
        cand = [b for b in range(N_CORES) if slots[b] < N_LOC]
        b = min(cand, key=lambda i: (loads[i], slots[i]))
        assign[c] = b
        loads[b] += counts[c]
        slots[b] += 1
    target = counts.sum() // N_CORES
    for _ in range(4096):
        hi = int(np.argmax(loads))
        lo = int(np.argmin(loads))
        if loads[hi] <= max(target, 128):
            break
        best = None
        ch = np.where(assign == hi)[0]
        cl = np.where(assign == lo)[0]
        for a in ch:
            for b2 in cl:
                d = counts[a] - counts[b2]
                if 0 < d <= loads[hi] - loads[lo]:
                    if best is None or abs(d - (loads[hi] - target)) < abs(
                        best[2] - (loads[hi] - target)
                    ):
                        best = (a, b2, d)
        if best is None:
            break
        a, b2, d = best
        assign[a], assign[b2] = lo, hi
        loads[hi] -= d
        loads[lo] += d
    return assign, loads


def _wrap_idxs(idx_flat, n_pad):
    """Pack flat gather indices into the dma_gather wrapped layout:
    [128, n_pad//16] int16 with logical index i at [i%16, i//16],
    replicated across the 8 groups of 16 partitions."""
    assert n_pad % 16 == 0
    w = np.zeros((16, n_pad // 16), dtype=np.int16)
    for i, v in enumerate(idx_flat):
        w[i % 16, i // 16] = v
    return np.tile(w, (8, 1))


def _prepare(x, W_agg, corner_edge_pairs, edge_corner, num_corners):
    x = np.asarray(x, dtype=np.float32)
    W_agg = np.asarray(W_agg, dtype=np.float32)
    cep = np.asarray(corner_edge_pairs).astype(np.int64)
    ec = np.asarray(edge_corner).astype(np.int64)
    ncorn = int(num_corners)
    assert x.shape == (E, C, H, W), x.shape
    assert ncorn <= NC_TOT

    # reference semantics: scatter drops out-of-range segments, gathers clamp
    seg = cep[:, 0]
    eid = np.clip(cep[:, 1], 0, E - 1)
    valid = (seg >= 0) & (seg < ncorn)
    seg_v, eid_v = seg[valid], eid[valid]
    ec_cl = np.clip(ec, 0, max(ncorn - 1, 0))

    counts = np.bincount(seg_v, minlength=NC_TOT).astype(np.int64)
    inv_count = 1.0 / np.maximum(counts, 1).astype(np.float64)

    assign, loads = _balance_corners(counts)
    k_chunks = max(1, int(-(-int(loads.max()) // 128)))  # ceil(maxload/128)
    k_pad = 128 * k_chunks

    # permuted corner position in the all-gathered table
    pos = np.zeros(NC_TOT, dtype=np.int64)
    slot_ctr = np.zeros(N_CORES, dtype=np.int64)
    for c in range(NC_TOT):
        b = assign[c]
        pos[c] = b * N_LOC + slot_ctr[b]
        slot_ctr[b] += 1

    # per-core incident pair lists
    pair_eids = [[] for _ in range(N_CORES)]
    pair_local = [[] for _ in range(N_CORES)]
    pair_inv = [[] for _ in range(N_CORES)]
    for p in range(len(seg_v)):
        c = int(seg_v[p])
        b = int(assign[c])
        pair_eids[b].append(int(eid_v[p]))
        pair_local[b].append(int(pos[c] - b * N_LOC))
        pair_inv[b].append(inv_count[c])

    # x in bf16 (local conv/max input) and fp8 (scatter gather source)
    xr = x.reshape(E, C, HW)
    xb = _round_bf16(xr)                                  # [E, 64, 784] u16
    x8 = np.clip(xr, -240.0, 240.0).astype(FP8)           # [E, 64, 784] fp8
    # bf16 local layouts, pre-shuffled so each core's chunk is one flat
    # [128, 32*CH] copy: partition p = (edge%2)*64 + ch, col = je*CH + d
    def _shuffle_local(xc):
        CH = xc.shape[2]
        # [E, 64, CH] -> per core [32 je, 2 m, 64 ch, CH] -> [(m ch), (je d)]
        a = xc.reshape(N_CORES, E_LOC // 2, 2, C, CH)
        a = a.transpose(0, 2, 3, 1, 4)            # [b, m, ch, je, d]
        return np.ascontiguousarray(a).reshape(N_CORES * 128, (E_LOC // 2) * CH)

    xl0 = _shuffle_local(xb[:, :, :CH0])
    xl1 = _shuffle_local(xb[:, :, CH0:])
    # fp8 gather layouts (big rows: fewer SWDGE descriptors)
    xf0 = np.ascontiguousarray(x8[:, :, :CH0]).reshape(E * 8, 8 * CH0)
    xf1 = np.ascontiguousarray(x8[:, :, CH0:]).reshape(E * 2, 32 * CH1)

    # block-diagonal weights for 2-edge batched conv matmuls
    wblk = np.zeros((4, 128, 128), dtype=np.float32)
    for t in range(4):
        wt = W_agg[:, t * 64:(t + 1) * 64].T          # [c, o]
        wblk[t, :64, :64] = wt
        wblk[t, 64:, 64:] = wt
    wblk_in = _round_bf16(wblk.reshape(4 * 128, 128))
    # W2/W3 block-diag stacked on the k-tile dim, fp8 for DoubleRow matmul
    w23 = np.stack([wblk[1], wblk[2]], axis=1)            # [128, 2, 128]
    w23_8 = np.clip(w23, -240.0, 240.0).astype(FP8).reshape(256, 128)
    ident = _round_bf16(np.eye(128, dtype=np.float32))

    per_core = []
    for b in range(N_CORES):
        k_real = len(pair_eids[b])
        assert k_real <= k_pad
        eids_b = np.zeros(k_pad, dtype=np.int64)
        eids_b[:k_real] = pair_eids[b]
        mc = np.zeros((k_pad, N_LOC), dtype=np.float32)
        for p in range(k_real):
            mc[p, pair_local[b][p]] += pair_inv[b][p]

        # stage-1 gather indices
        # chunk0: per (kc, j in 8): 128 idxs, idx = eid*8 + j  (8 chs/row)
        # chunk1: per (kc, g in 2): 128 idxs, idx = eid*2 + g  (32 chs/row)
        s10_cols = []
        s11_cols = []
        for kc in range(k_chunks):
            epk = eids_b[kc * 128:(kc + 1) * 128]
            for j in range(8):
                s10_cols.append(_wrap_idxs((epk * 8 + j).astype(np.int16), 128))
            for g in range(2):
                s11_cols.append(_wrap_idxs((epk * 2 + g).astype(np.int16), 128))
        s10 = np.concatenate(s10_cols, axis=1)   # [128, 8*8*k_chunks]
        s11 = np.concatenate(s11_cols, axis=1)   # [128, 8*2*k_chunks]

        # stage-4 gather indices: per binstr, 1024 idxs into the ag table
        # i = s*128 + m*64 + ch -> table row of corner(edge, t), channel ch
        e0 = b * E_LOC
        s4_cols = []
        ch64 = np.arange(64, dtype=np.int64)
        for binstr in range(8):
            f = np.zeros(1024, dtype=np.int64)
            for ep in range(4):
                for t in range(2):
                    s = ep * 2 + t
                    for m in range(2):
                        le = binstr * 8 + ep * 2 + m
                        p_c = pos[int(ec_cl[e0 + le, t])]
                        ob, sl = p_c // N_LOC, p_c % N_LOC
                        i0 = s * 128 + m * 64
                        f[i0:i0 + 64] = ob * SROWS + sl * 64 + ch64
            s4_cols.append(_wrap_idxs(f.astype(np.int16), 1024))
        s4 = np.concatenate(s4_cols, axis=1)     # [128, 512]

        per_core.append(dict(mc=_round_bf16(mc), s10=s10, s11=s11, s4=s4))

    return xl0, xl1, xf0, xf1, wblk_in, w23_8, ident, per_core, k_chunks


# --------------------------------------------------------------------------
# device program
# --------------------------------------------------------------------------

def _build_program(k_chunks):
    dbg_no_prep = os.environ.get("DBG_NO_PREP", "0") == "1"
    dbg_zero_table = os.environ.get("DBG_ZERO_TABLE", "0") == "1"
    dbg_zero_xw = os.environ.get("DBG_ZERO_XW", "0") == "1"
    bf = mybir.dt.bfloat16
    f8 = mybir.dt.float8e4
    f32 = mybir.dt.float32
    i16 = mybir.dt.int16

    nc = bacc.Bacc("TRN2", target_bir_lowering=False, debug=False,
                   num_devices=N_CORES, num_swdge_queues=2)

    xf0_t = nc.dram_tensor("xf0", [E * 8, 8 * CH0], f8, kind="ExternalInput").ap()
    xf1_t = nc.dram_tensor("xf1", [E * 2, 32 * CH1], f8, kind="ExternalInput").ap()
    xl0_t = nc.dram_tensor("xl0", [128, 32 * CH0], bf, kind="ExternalInput").ap()
    xl1_t = nc.dram_tensor("xl1", [128, 32 * CH1], bf, kind="ExternalInput").ap()
    wb_t = nc.dram_tensor("wb", [4 * 128, 128], bf, kind="ExternalInput").ap()
    w23_t = nc.dram_tensor("w23", [256, 128], f8, kind="ExternalInput").ap()
    id_t = nc.dram_tensor("ident", [128, 128], bf, kind="ExternalInput").ap()
    mc_t = nc.dram_tensor("mc", [128 * k_chunks, N_LOC], bf, kind="ExternalInput").ap()
    s10_t = nc.dram_tensor("s10", [128, 8 * 8 * k_chunks], i16, kind="ExternalInput").ap()
    s11_t = nc.dram_tensor("s11", [128, 8 * 2 * k_chunks], i16, kind="ExternalInput").ap()
    s4_t = nc.dram_tensor("s4", [128, 512], i16, kind="ExternalInput").ap()
    out_t = nc.dram_tensor("outp", [E_LOC * C, HW], bf, kind="ExternalOutput").ap()
    dbg_dump = os.environ.get("DBG_DUMP_TABLE", "0") != "0"
    if dbg_dump:
        dump_t = nc.dram_tensor("dump", [N_CORES * SROWS, TCOLS], mybir.dt.uint8,
                                kind="ExternalOutput").ap()

    with tile.TileContext(nc) as tc:
        with tc.tile_pool(name="dram", bufs=1, space="DRAM") as dram, \
             tc.tile_pool(name="consts", bufs=1) as consts, \
             tc.tile_pool(name="xw", bufs=1) as xwp:
            slice_t = dram.tile([SROWS, TCOLS], f8)
            ag_t = dram.tile([N_CORES * SROWS, TCOLS], f8, addr_space="Shared")

            # constants (one DMA each)
            wtile = consts.tile([128, 4, 128], bf, tag="wt")
            nc.sync.dma_start(out=wtile[:],
                              in_=wb_t[:].rearrange("(t p) o -> p t o", t=4))
            w23tile = consts.tile([128, 2, 128], f8, tag="w23")
            nc.sync.dma_start(out=w23tile[:],
                              in_=w23_t[:].rearrange("(p t) o -> p t o", t=2))
            mctiles = []
            for kc in range(k_chunks):
                mt = consts.tile([128, N_LOC], bf, tag=f"mc{kc}")
                nc.sync.dma_start(out=mt[:], in_=mc_t[kc * 128:(kc + 1) * 128, :])
                mctiles.append(mt)
            s10tile = consts.tile([128, 8 * 8 * k_chunks], i16, tag="s10")
            nc.sync.dma_start(out=s10tile[:], in_=s10_t[:])
            s11tile = consts.tile([128, 8 * 2 * k_chunks], i16, tag="s11")
            nc.sync.dma_start(out=s11tile[:], in_=s11_t[:])
            s4tile = consts.tile([128, 512], i16, tag="s4")
            nc.sync.dma_start(out=s4tile[:], in_=s4_t[:])
            gm0 = consts.tile([128, CH0], bf, tag="gm0")
            gm1 = consts.tile([128, CH1], bf, tag="gm1")

            # zero the table pad columns (cols HW..TCOLS) and gmax rows once
            PAD = TCOLS - HW
            zt = consts.tile([128, (S0R // 128) * PAD], mybir.dt.float8e4,
                             tag="zt")
            nc.vector.memset(zt[:], 0.0)
            nc.sync.dma_start(
                out=slice_t[0:S0R, :].rearrange("(g r) d -> r g d", r=128)
                [:, :, HW:TCOLS],
                in_=zt[:],
            )
            nc.sync.dma_start(out=slice_t[S0R:SROWS, :],
                              in_=zt[:, 0:TCOLS])

            # x stays resident in SBUF; phase 4 reads it as matmul rhs
            xb0 = xwp.tile([128, 32 * CH0], bf, tag="xb0")
            xb1 = xwp.tile([128, 32 * CH1], bf, tag="xb1")

            def w(t):
                return wtile[:, t, :]

            # phase-4 gather tiles: own up-front pool so the prepped DMA
            # writes never land in SBUF space being reused by other pools
            lrts = []
            for binstr in range(8):
                lrt = xwp.tile([128, 8, TCOLS], f8, tag=f"lrt{binstr}")
                lrts.append(lrt)

            with tc.tile_pool(name="p2", bufs=1) as p2, \
                 tc.tile_pool(name="p2r", bufs=8) as p2r, \
                 tc.tile_pool(name="p1", bufs=4) as p1, \
                 tc.tile_pool(name="p1b", bufs=3) as p1b, \
                 tc.tile_pool(name="p1s", bufs=6) as p1s, \
                 tc.tile_pool(name="psum1", bufs=3, space="PSUM") as psum1, \
                 tc.tile_pool(name="psum1b", bufs=5, space="PSUM") as psum1b:
                mx0 = p2.tile([128, CH0], bf, tag="mx0")
                mx1 = p2.tile([128, CH1], bf, tag="mx1")

                def scatter0(j):
                    gts = []
                    for kc in range(k_chunks):
                        gt = p1.tile([128, 1, 8 * CH0], f8, tag="gt0")
                        nc.gpsimd.dma_gather(
                            gt[:], xf0_t[:],
                            s10tile[:, (kc * 8 + j) * 8:(kc * 8 + j) * 8 + 8],
                            num_idxs=128, num_idxs_reg=128, elem_size=8 * CH0,
                        )
                        gts.append(gt)
                    for t in range(2):
                        ps = psum1.tile([128, CH0], f32, space="PSUM", tag="ps1a")
                        for i in range(4):
                            q = t * 4 + i
                            for kc in range(k_chunks):
                                nc.tensor.matmul(
                                    out=ps[32 * i:32 * (i + 1), :],
                                    lhsT=mctiles[kc][:],
                                    rhs=gts[kc][:, 0, q * CH0:(q + 1) * CH0],
                                    start=(kc == 0), stop=(kc == k_chunks - 1),
                                    tile_position=(0, 32 * i),
                                )
                        stg = p1s.tile([128, CH0], f8, tag="stg0")
                        nc.scalar.activation(stg[:], ps[:],
                                             mybir.ActivationFunctionType.Copy)
                        nc.sync.dma_start(
                            out=slice_t[0:S0R, :]
                            .rearrange("(c h) d -> h c d", c=N_LOC)
                            [8 * j + 4 * t:8 * j + 4 * t + 4, :, 0:CH0],
                            in_=stg[:],
                        )

                def scatter1(g):
                    gts = []
                    for kc in range(k_chunks):
                        gt = p1b.tile([128, 1, 32 * CH1], f8, tag="gt1")
                        nc.gpsimd.dma_gather(
                            gt[:], xf1_t[:],
                            s11tile[:, (kc * 2 + g) * 8:(kc * 2 + g) * 8 + 8],
                            num_idxs=128, num_idxs_reg=128, elem_size=32 * CH1,
                        )
                        gts.append(gt)
                    for t in range(8):
                        ps = psum1b.tile([128, CH1], f32, space="PSUM", tag="ps1b")
                        for i in range(4):
                            q = t * 4 + i
                            for kc in range(k_chunks):
                                nc.tensor.matmul(
                                    out=ps[32 * i:32 * (i + 1), :],
                                    lhsT=mctiles[kc][:],
                                    rhs=gts[kc][:, 0, q * CH1:(q + 1) * CH1],
                                    start=(kc == 0), stop=(kc == k_chunks - 1),
                                    tile_position=(0, 32 * i),
                                )
                        stg = p1s.tile([128, CH1], f8, tag="stg1")
                        nc.scalar.activation(stg[:], ps[:],
                                             mybir.ActivationFunctionType.Copy)
                        nc.sync.dma_start(
                            out=slice_t[0:S0R, :]
                            .rearrange("(c h) d -> h c d", c=N_LOC)
                            [32 * g + 4 * t:32 * g + 4 * t + 4, :,
                             CH0:CH0 + CH1],
                            in_=stg[:],
                        )

                # x loads up front; chunk1 scatter (independent of x)
                # interleaved mid-stream; max chains ride the vector queue
                nc.scalar.dma_start(out=xb0[:], in_=xl0_t[:])
                nc.scalar.dma_start(out=xb1[:], in_=xl1_t[:])
                scatter1(0)
                scatter0(0)
                scatter0(1)
                scatter0(2)
                scatter1(1)
                scatter0(3)
                scatter0(4)
                scatter0(5)
                scatter0(6)
                scatter0(7)
                for chunk, mx, xc, CH in ((0, mx0, xb0, CH0), (1, mx1, xb1, CH1)):
                    for j in range(32):
                        sl = xc[:, j * CH:(j + 1) * CH]
                        if j == 0:
                            nc.vector.tensor_copy(out=mx[:], in_=sl)
                        else:
                            nc.vector.tensor_tensor(out=mx[:], in0=mx[:],
                                                    in1=sl,
                                                    op=mybir.AluOpType.max)

                # ---- local max fold -> gmax rows of the table slice -------
                half0 = p2.tile([64, CH0], bf, tag="h0")
                nc.scalar.dma_start(out=half0[:], in_=mx0[64:128, :])
                nc.vector.tensor_tensor(out=mx0[0:64, :], in0=mx0[0:64, :],
                                        in1=half0[:], op=mybir.AluOpType.max)
                half1 = p2.tile([64, CH1], bf, tag="h1")
                nc.scalar.dma_start(out=half1[:], in_=mx1[64:128, :])
                nc.vector.tensor_tensor(out=mx1[0:64, :], in0=mx1[0:64, :],
                                        in1=half1[:], op=mybir.AluOpType.max)
                # gmax bf16 bytes ride the fp8 table AllGather: per channel c,
                # row S0R+2c = cols 0:511 (1024B), row S0R+2c+1 = cols 512:783
                nc.sync.dma_start(
                    out=slice_t[S0R:SROWS, :]
                    .rearrange("(c two) d -> c (two d)", two=2)[:, 0:2 * CH0],
                    in_=mx0[0:64, :].bitcast(f8),
                )
                nc.sync.dma_start(
                    out=slice_t[S0R:SROWS, :]
                    .rearrange("(c two) d -> c (two d)", two=2)
                    [:, TCOLS:TCOLS + 2 * CH1],
                    in_=mx1[0:64, :].bitcast(f8),
                )

                # ---- phase-4 gather preps + AllGather, all on gpsimd:
                # NPRE preps generate during the phase-1 tail, then the AG
                # doorbell rings (it blocks gpsimd until the AG completes),
                # trigger #1 fires the first wave the moment the AG lands,
                # and the remaining preps + trigger #2 follow ---------------
                NPRE = 8
                p4sems = []
                preps = []

                def emit_prep(binstr):
                    dma_sem = nc.alloc_semaphore(f"p4g{binstr}")
                    pr = nc.gpsimd.dma_gather(
                        lrts[binstr][:], ag_t[:],
                        s4tile[:, binstr * 64:(binstr + 1) * 64],
                        num_idxs=1024, num_idxs_reg=1024, elem_size=TCOLS,
                        prepare_only=True, sem=dma_sem, queue_num=1,
                    )
                    preps.append(pr)
                    p4sems.append(dma_sem)

                if not dbg_no_prep:
                    for binstr in range(NPRE):
                        emit_prep(binstr)
                agi = nc.gpsimd.collective_compute(
                    "AllGather", mybir.AluOpType.bypass,
                    replica_groups=[list(range(N_CORES))],
                    ins=[slice_t.opt()], outs=[ag_t.opt()],
                )
                trig = None
                if not dbg_no_prep:
                    for pr in preps:
                        bass._add_dep_helper(agi.ins, pr.ins, False,
                                             "pin AG bell after prep wave 1")
                    trig = nc.gpsimd.trigger_dma(count=None, queue_num=1)
                    bass._add_dep_helper(trig.ins, agi.ins, True,
                                         "gather DMA fires after AllGather")
                    if NPRE < 8:
                        for binstr in range(NPRE, 8):
                            emit_prep(binstr)
                            bass._add_dep_helper(preps[-1].ins, agi.ins, False,
                                                 "pin prep wave 2 after AG bell")
                        trig2 = nc.gpsimd.trigger_dma(count=None, queue_num=1)
                        bass._add_dep_helper(trig2.ins, agi.ins, True,
                                             "gather DMA fires after AllGather")

                # ---- global max: parallel block loads + tree reduce -------
                t0s, t1s = [], []
                for b in range(N_CORES):
                    gv = ag_t[b * SROWS + S0R:(b * SROWS) + SROWS, :] \
                        .rearrange("(c two) d -> c (two d)", two=2)
                    t0 = p2r.tile([64, CH0], bf, tag="t0")
                    ld0 = nc.sync.dma_start(out=t0[:],
                                            in_=gv[:, 0:2 * CH0].bitcast(bf))
                    bass._add_dep_helper(ld0.ins, agi.ins, True,
                                         "gmax block read after AllGather")
                    t1 = p2r.tile([64, CH1], bf, tag="t1")
                    ld1 = nc.sync.dma_start(
                        out=t1[:],
                        in_=gv[:, TCOLS:TCOLS + 2 * CH1].bitcast(bf))
                    bass._add_dep_helper(ld1.ins, agi.ins, True,
                                         "gmax block read after AllGather")
                    t0s.append(t0)
                    t1s.append(t1)
                for ts in (t0s, t1s):
                    for stride in (4, 2, 1):
                        for i in range(stride):
                            nc.vector.tensor_tensor(
                                out=ts[i][:], in0=ts[i][:],
                                in1=ts[i + stride][:],
                                op=mybir.AluOpType.max)
                nc.sync.dma_start(out=gm0[0:64, :], in_=t0s[0][:])
                nc.sync.dma_start(out=gm0[64:128, :], in_=t0s[0][:])
                nc.sync.dma_start(out=gm1[0:64, :], in_=t1s[0][:])
                nc.sync.dma_start(out=gm1[64:128, :], in_=t1s[0][:])

            if dbg_dump:
                if os.environ.get("DBG_DUMP_TABLE") == "2":
                    nc.sync.dma_start(out=dump_t[0:S0R, :],
                                      in_=slice_t[:].bitcast(mybir.dt.uint8))
                else:
                    nc.sync.dma_start(out=dump_t[:],
                                      in_=ag_t[:].bitcast(mybir.dt.uint8))

            # ---- phase 4: prep gathers during AG, trigger after -----------
            with tc.tile_pool(name="p4o", bufs=4) as p4o, \
                 tc.tile_pool(name="psum4", bufs=4, space="PSUM") as psum4:
                if dbg_no_prep:
                    for binstr in range(8):
                        gi = nc.gpsimd.dma_gather(
                            lrts[binstr][:], ag_t[:],
                            s4tile[:, binstr * 64:(binstr + 1) * 64],
                            num_idxs=1024, num_idxs_reg=1024, elem_size=TCOLS,
                        )
                        bass._add_dep_helper(gi.ins, agi.ins, True,
                                             "gather reads table after AG")

                for binstr in range(8):
                    lrt = lrts[binstr]
                    wt = None
                    if trig is not None:
                        wt = nc.tensor.wait_ge(p4sems[binstr], 16)
                        bass._add_dep_helper(
                            wt.ins, trig.ins, True,
                            "phase4 lrt consume waits gather DMA (post-trigger)")
                    ot0 = p4o.tile([128, 4, CH0], bf, tag="ot0")
                    ot1 = p4o.tile([128, 4, CH1], bf, tag="ot1")
                    for ep in range(4):
                        je = binstr * 4 + ep
                        for half in range(2):
                            CH = CH0 if half == 0 else CH1
                            c0 = 0 if half == 0 else CH0
                            xk = xb0 if half == 0 else xb1
                            ot = ot0 if half == 0 else ot1
                            ps = psum4.tile([128, CH], f32, space="PSUM",
                                            tag=f"ps4{half}")
                            gm = gm0 if half == 0 else gm1
                            mms = []
                            if not dbg_zero_table:
                                # W2 @ CF[l] + W3 @ CF[r]: one fp8 DoubleRow
                                # matmul over the 2 stacked k-tiles
                                mms.append((dict(
                                    lhsT=w23tile[:],
                                    rhs=lrt[:, ep * 2:ep * 2 + 2, c0:c0 + CH],
                                    perf_mode=mybir.MatmulPerfMode.DoubleRow),
                                    True))
                            if not dbg_zero_xw:
                                mms.append((dict(
                                    lhsT=w(0),
                                    rhs=xk[:, je * CH:(je + 1) * CH]),
                                    False))
                                mms.append((dict(lhsT=w(3), rhs=gm[:]), False))
                            for k, (kw, is_lrt) in enumerate(mms):
                                mm = nc.tensor.matmul(
                                    out=ps[:], start=(k == 0),
                                    stop=(k == len(mms) - 1), **kw)
                                if wt is not None and is_lrt:
                                    bass._add_dep_helper(
                                        mm.ins, wt.ins, True,
                                        "mm reads lrt after DMA wait")
                            nc.scalar.activation(
                                ot[:, ep, :], ps[:],
                                mybir.ActivationFunctionType.Relu)

                    # out row (binstr*8 + ep*2)*64 + p = bi*512 + ep*128 + p
                    nc.sync.dma_start(
                        out=out_t[:]
                        .rearrange("(bi ep p) d -> bi p ep d", bi=8, ep=4)
                        [binstr, :, :, 0:CH0],
                        in_=ot0[:],
                    )
                    nc.sync.dma_start(
                        out=out_t[:]
                        .rearrange("(bi ep p) d -> bi p ep d", bi=8, ep=4)
                        [binstr, :, :, CH0:HW],
                        in_=ot1[:],
                    )


    nc.compile()
    return nc


# --------------------------------------------------------------------------
# entry point
# --------------------------------------------------------------------------

def _run(x, W_agg, corner_edge_pairs, edge_corner, num_corners,
         trace=False):
    xl0, xl1, xf0, xf1, wblk_in, w23_8, ident, per_core, k_chunks = _prepare(
        x, W_agg, corner_edge_pairs, edge_corner, num_corners)

    key = (k_chunks, os.environ.get("DBG_NO_PREP"),
           os.environ.get("DBG_ZERO_TABLE"), os.environ.get("DBG_ZERO_XW"))
    if key not in _PROGRAM_CACHE:
        _PROGRAM_CACHE[key] = _build_program(k_chunks)
    nc = _PROGRAM_CACHE[key]

    xl0_b = xl0.view(BF16)
    xl1_b = xl1.view(BF16)
    in_maps = []
    for b in range(N_CORES):
        pc = per_core[b]
        in_maps.append({
            "xf0": xf0, "xf1": xf1,
            "xl0": xl0_b[b * 128:(b + 1) * 128],
            "xl1": xl1_b[b * 128:(b + 1) * 128],
            "wb": wblk_in.view(BF16), "w23": w23_8, "ident": ident.view(BF16),
            "mc": pc["mc"].view(BF16),
            "s10": pc["s10"], "s11": pc["s11"], "s4": pc["s4"],
        })

    kwargs = {}
    if trace:
        kwargs = dict(trace=True, trace_cores=list(range(N_CORES)))
    res = run_bass_kernel_spmd(nc, in_maps, list(range(N_CORES)), **kwargs)

    out = np.empty((E, C, HW), dtype=np.float32)
    for b in range(N_CORES):
        o = np.asarray(res.results[b]["outp"]).view(np.uint16)
        f = (o.astype(np.uint32) << 16).view(np.float32).reshape(E_LOC, C, HW)
        out[b * E_LOC:(b + 1) * E_LOC] = f
    return out.reshape(E, C, H, W), res


def kernel(x, W_agg, corner_edge_pairs, edge_corner, num_corners):
    out, _ = _run(x, W_agg, corner_edge_pairs, edge_corner, num_corners,
                  trace=False)
    return out


# expose for test harness profiling
def _run_profiled(x, W_agg, corner_edge_pairs, edge_corner, num_corners,
                  trace=True):
    return _run(x, W_agg, corner_edge_pairs, edge_corner, num_corners,
                trace=trace)
